# revision 7
# baseline (speedup 1.0000x reference)
"""MultiInnerProductDecoder2 Trainium2 Bass kernel.

reference math:
    src, dst = edge_index[:, 0, :], edge_index[:, 1, :]           # [T, E]
    vals = einsum('ted,ted,td->te', z[src], z[dst], weight)       # [T, E]
    per_type = sigmoid(vals)                                      # [T, E]
    score    = sigmoid(vals.reshape(-1))                          # [T*E]

Strategy: pure data parallel over edge types — 16 types / 8 cores = 2 types
per core, z (51MB) and per-core weight rows replicated. Per-edge gather of
z rows via gpsimd indirect DMA (512B row descriptors), product + weighted
reduce on DVE, sigmoid on ACT.

Edge blocking: per type, the E edges are processed as a [128, W] blocked
layout (edge e = p*W + j maps to partition p, column j) so both the index
load and the value store are large contiguous DMAs. The src/dst indices are
host-interleaved to [E, 2] so ONE indirect gather per group fetches both
endpoints of each edge.
"""

import numpy as np

import concourse.bacc as bacc
import concourse.bass as bass
import concourse.mybir as mybir
import concourse.tile as tile
from concourse.bass_utils import run_bass_kernel_spmd

N_NODES = 100000
D = 128
NUM_ET = 16
E = 100000
N_CORES = 8
TPC = NUM_ET // N_CORES  # edge types per core
K = 8  # 128-edge columns per gather group

F32 = mybir.dt.float32
I32 = mybir.dt.int32


def build_program(
    n_nodes=N_NODES, d=D, tpc=TPC, e=E, k=K, num_devices=N_CORES, iters=1
):
    nc = bacc.Bacc(
        "TRN2", target_bir_lowering=False, debug=False, num_devices=num_devices
    )
    z_t = nc.dram_tensor("z", [n_nodes, d], F32, kind="ExternalInput")
    wb_t = nc.dram_tensor("wb", [tpc, 128, d], F32, kind="ExternalInput")
    idx_t = nc.dram_tensor("idx", [tpc, 2 * e], I32, kind="ExternalInput")
    out_t = nc.dram_tensor("out", [tpc, e], F32, kind="ExternalOutput")

    w_cols = e // 128  # main-block columns
    e_main = 128 * w_cols
    tail = e - e_main
    assert tail != 1, "single-element tail not supported"

    mult = mybir.AluOpType.mult
    add = mybir.AluOpType.add
    sig = mybir.ActivationFunctionType.Sigmoid

    with tile.TileContext(nc) as tc:
        with (
            tc.tile_pool(name="pidx", bufs=2) as pidx,
            tc.tile_pool(name="pz", bufs=3) as pz,
            tc.tile_pool(name="pw", bufs=2) as pw,
            tc.tile_pool(name="pvals", bufs=2) as pvals,
            tc.tile_pool(name="psmall", bufs=3) as psmall,
        ):
            import contextlib

            loop_ctx = tc.For_i(0, iters, 1) if iters > 1 else contextlib.nullcontext()
            with loop_ctx:
                _emit_body(
                    nc, tc, z_t, wb_t, idx_t, out_t,
                    pidx, pz, pw, pvals, psmall,
                    tpc, d, e, k, w_cols, e_main, tail, mult, add, sig,
                )
    nc.compile()
    return nc


def _emit_body(
    nc, tc, z_t, wb_t, idx_t, out_t,
    pidx, pz, pw, pvals, psmall,
    tpc, d, e, k, w_cols, e_main, tail, mult, add, sig,
):
    if True:
        if True:
            for s in range(tpc):
                wb_tile = pw.tile([128, d], F32, tag="wb")
                nc.sync.dma_start(out=wb_tile[:], in_=wb_t[s])

                if w_cols > 0:
                    idx_tile = pidx.tile([128, 2 * w_cols], I32, tag="idx")
                    nc.sync.dma_start(
                        out=idx_tile[:],
                        in_=idx_t[s, : 2 * e_main].rearrange(
                            "(p w) -> p w", w=2 * w_cols
                        ),
                    )
                    vals_tile = pvals.tile([128, w_cols], F32, tag="vals")

                    for j0 in range(0, w_cols, k):
                        kk = min(k, w_cols - j0)
                        zpair = pz.tile([128, 2 * k * d], F32, tag="zpair")
                        # HW indirect DMA supports ONE offset per partition:
                        # one gather per (column, side), each fetching 128 rows
                        # into a [128, d] slice of the group tile.
                        for q in range(2 * kk):
                            nc.gpsimd.indirect_dma_start(
                                out=zpair[:, q * d : (q + 1) * d],
                                out_offset=None,
                                in_=z_t.ap(),
                                in_offset=bass.IndirectOffsetOnAxis(
                                    ap=idx_tile[:, 2 * j0 + q : 2 * j0 + q + 1],
                                    axis=0,
                                ),
                            )
                        v3 = zpair[:, : 2 * kk * d].rearrange(
                            "p (kk s d) -> p kk s d", s=2, d=d
                        )
                        src = v3[:, :, 0, :]
                        dst = v3[:, :, 1, :]
                        nc.vector.tensor_tensor(out=src, in0=src, in1=dst, op=mult)
                        wbb = wb_tile[:].unsqueeze(1).to_broadcast([128, kk, d])
                        nc.vector.tensor_tensor(out=src, in0=src, in1=wbb, op=mult)
                        red = psmall.tile([128, k], F32, tag="red")
                        nc.vector.tensor_reduce(
                            out=red[:, :kk], in_=src, axis=mybir.AxisListType.X, op=add
                        )
                        nc.scalar.activation(
                            out=vals_tile[:, j0 : j0 + kk], in_=red[:, :kk], func=sig
                        )

                    nc.sync.dma_start(
                        out=out_t[s, :e_main].rearrange("(p w) -> p w", w=w_cols),
                        in_=vals_tile[:],
                    )

                if tail:
                    ti = psmall.tile([tail, 2], I32, tag="tidx")
                    nc.sync.dma_start(
                        out=ti[:],
                        in_=idx_t[s, 2 * e_main : 2 * e].rearrange("(p w) -> p w", w=2),
                    )
                    zt = psmall.tile([tail, 2 * d], F32, tag="ztail")
                    for q in range(2):
                        nc.gpsimd.indirect_dma_start(
                            out=zt[:, q * d : (q + 1) * d],
                            out_offset=None,
                            in_=z_t.ap(),
                            in_offset=bass.IndirectOffsetOnAxis(
                                ap=ti[:, q : q + 1], axis=0
                            ),
                        )
                    v3t = zt[:].rearrange("p (s d) -> p s d", d=d)
                    srct = v3t[:, 0, :]
                    dstt = v3t[:, 1, :]
                    nc.vector.tensor_tensor(out=srct, in0=srct, in1=dstt, op=mult)
                    nc.vector.tensor_tensor(
                        out=srct, in0=srct, in1=wb_tile[:tail, :], op=mult
                    )
                    redt = psmall.tile([tail, 1], F32, tag="redt")
                    nc.vector.tensor_reduce(
                        out=redt[:], in_=srct, axis=mybir.AxisListType.X, op=add
                    )
                    sigt = psmall.tile([tail, 1], F32, tag="sigt")
                    nc.scalar.activation(out=sigt[:], in_=redt[:], func=sig)
                    nc.sync.dma_start(
                        out=out_t[s, e_main:e].rearrange("(p w) -> p w", w=1),
                        in_=sigt[:],
                    )
    return nc


_PROGRAM = None


def _get_program():
    global _PROGRAM
    if _PROGRAM is None:
        _PROGRAM = build_program()
    return _PROGRAM


def make_in_maps(z, weight, edge_index):
    z = np.ascontiguousarray(np.asarray(z, dtype=np.float32))
    weight = np.asarray(weight, dtype=np.float32)
    ei = np.asarray(edge_index)
    t, _, e = ei.shape
    idx32 = ei.astype(np.int32).transpose(0, 2, 1)  # [T, E, 2] (edge-interleaved)
    wb = np.broadcast_to(weight[:, None, :], (t, 128, weight.shape[1]))
    in_maps = []
    for c in range(N_CORES):
        sl = slice(c * TPC, (c + 1) * TPC)
        in_maps.append(
            {
                "z": z,
                "wb": np.ascontiguousarray(wb[sl]),
                "idx": np.ascontiguousarray(idx32[sl]).reshape(TPC, 2 * e),
            }
        )
    return in_maps


def kernel(z, weight, edge_index, _trace=False):
    nc = _get_program()
    in_maps = make_in_maps(z, weight, edge_index)
    res = run_bass_kernel_spmd(
        nc, in_maps, core_ids=list(range(N_CORES)), trace=_trace
    )
    per_type = np.concatenate([res.results[c]["out"] for c in range(N_CORES)], axis=0)
    score = np.ascontiguousarray(per_type.reshape(-1))
    if _trace:
        kernel._last_results = res
    return per_type, score


# revision 8
# speedup vs baseline: 1.0213x; 1.0213x over previous
"""MultiInnerProductDecoder2 Trainium2 Bass kernel.

reference math:
    src, dst = edge_index[:, 0, :], edge_index[:, 1, :]           # [T, E]
    vals = einsum('ted,ted,td->te', z[src], z[dst], weight)       # [T, E]
    per_type = sigmoid(vals)                                      # [T, E]
    score    = sigmoid(vals.reshape(-1))                          # [T*E]

Strategy: pure data parallel over edge types — 16 types / 8 cores = 2 types
per core, z (51MB) and per-core weight rows replicated. Per-edge gather of
z rows via gpsimd indirect DMA (512B row descriptors), product + weighted
reduce on DVE, sigmoid on ACT.

Edge blocking: per type, the E edges are processed as a [128, W] blocked
layout (edge e = p*W + j maps to partition p, column j) so both the index
load and the value store are large contiguous DMAs. The src/dst indices are
host-interleaved to [E, 2] so ONE indirect gather per group fetches both
endpoints of each edge.
"""

import numpy as np

import concourse.bacc as bacc
import concourse.bass as bass
import concourse.mybir as mybir
import concourse.tile as tile
from concourse.bass_utils import run_bass_kernel_spmd

N_NODES = 100000
D = 128
NUM_ET = 16
E = 100000
N_CORES = 8
TPC = NUM_ET // N_CORES  # edge types per core
K = 8  # 128-edge columns per gather group

F32 = mybir.dt.float32
I32 = mybir.dt.int32


def build_program(
    n_nodes=N_NODES, d=D, tpc=TPC, e=E, k=K, num_devices=N_CORES, iters=1
):
    nc = bacc.Bacc(
        "TRN2", target_bir_lowering=False, debug=False, num_devices=num_devices
    )
    z_t = nc.dram_tensor("z", [n_nodes, d], F32, kind="ExternalInput")
    wb_t = nc.dram_tensor("wb", [tpc, 128, d], F32, kind="ExternalInput")
    idx_t = nc.dram_tensor("idx", [tpc, 2 * e], I32, kind="ExternalInput")
    out_t = nc.dram_tensor("out", [tpc, e], F32, kind="ExternalOutput")

    w_cols = e // 128  # main-block columns
    e_main = 128 * w_cols
    tail = e - e_main
    assert tail != 1, "single-element tail not supported"

    mult = mybir.AluOpType.mult
    add = mybir.AluOpType.add
    sig = mybir.ActivationFunctionType.Sigmoid

    with tile.TileContext(nc) as tc:
        with (
            tc.tile_pool(name="pidx", bufs=2) as pidx,
            tc.tile_pool(name="pz", bufs=3) as pz,
            tc.tile_pool(name="pw", bufs=2) as pw,
            tc.tile_pool(name="pvals", bufs=2) as pvals,
            tc.tile_pool(name="psmall", bufs=3) as psmall,
        ):
            import contextlib

            loop_ctx = tc.For_i(0, iters, 1) if iters > 1 else contextlib.nullcontext()
            with loop_ctx:
                _emit_body(
                    nc, tc, z_t, wb_t, idx_t, out_t,
                    pidx, pz, pw, pvals, psmall,
                    tpc, d, e, k, w_cols, e_main, tail, mult, add, sig,
                )
    nc.compile()
    return nc


def _emit_body(
    nc, tc, z_t, wb_t, idx_t, out_t,
    pidx, pz, pw, pvals, psmall,
    tpc, d, e, k, w_cols, e_main, tail, mult, add, sig,
):
    if True:
        if True:
            for s in range(tpc):
                wb_tile = pw.tile([128, d], F32, tag="wb")
                nc.sync.dma_start(out=wb_tile[:], in_=wb_t[s])

                if w_cols > 0:
                    idx_tile = pidx.tile([128, 2 * w_cols], I32, tag="idx")
                    nc.sync.dma_start(
                        out=idx_tile[:],
                        in_=idx_t[s, : 2 * e_main].rearrange(
                            "(p w) -> p w", w=2 * w_cols
                        ),
                    )
                    vals_tile = pvals.tile([128, w_cols], F32, tag="vals")

                    for j0 in range(0, w_cols, k):
                        kk = min(k, w_cols - j0)
                        zpair = pz.tile([128, 2 * k * d], F32, tag="zpair")
                        # HW indirect DMA supports ONE offset per partition:
                        # one gather per (column, side), each fetching 128 rows
                        # into a [128, d] slice of the group tile.
                        for q in range(2 * kk):
                            nc.gpsimd.indirect_dma_start(
                                out=zpair[:, q * d : (q + 1) * d],
                                out_offset=None,
                                in_=z_t.ap(),
                                oob_is_err=False,
                                in_offset=bass.IndirectOffsetOnAxis(
                                    ap=idx_tile[:, 2 * j0 + q : 2 * j0 + q + 1],
                                    axis=0,
                                ),
                            )
                        v3 = zpair[:, : 2 * kk * d].rearrange(
                            "p (kk s d) -> p kk s d", s=2, d=d
                        )
                        src = v3[:, :, 0, :]
                        dst = v3[:, :, 1, :]
                        nc.vector.tensor_tensor(out=src, in0=src, in1=dst, op=mult)
                        wbb = wb_tile[:].unsqueeze(1).to_broadcast([128, kk, d])
                        nc.vector.tensor_tensor(out=src, in0=src, in1=wbb, op=mult)
                        red = psmall.tile([128, k], F32, tag="red")
                        nc.vector.tensor_reduce(
                            out=red[:, :kk], in_=src, axis=mybir.AxisListType.X, op=add
                        )
                        nc.scalar.activation(
                            out=vals_tile[:, j0 : j0 + kk], in_=red[:, :kk], func=sig
                        )

                    nc.sync.dma_start(
                        out=out_t[s, :e_main].rearrange("(p w) -> p w", w=w_cols),
                        in_=vals_tile[:],
                    )

                if tail:
                    ti = psmall.tile([tail, 2], I32, tag="tidx")
                    nc.sync.dma_start(
                        out=ti[:],
                        in_=idx_t[s, 2 * e_main : 2 * e].rearrange("(p w) -> p w", w=2),
                    )
                    zt = psmall.tile([tail, 2 * d], F32, tag="ztail")
                    for q in range(2):
                        nc.gpsimd.indirect_dma_start(
                            out=zt[:, q * d : (q + 1) * d],
                            out_offset=None,
                            in_=z_t.ap(),
                            oob_is_err=False,
                            in_offset=bass.IndirectOffsetOnAxis(
                                ap=ti[:, q : q + 1], axis=0
                            ),
                        )
                    v3t = zt[:].rearrange("p (s d) -> p s d", d=d)
                    srct = v3t[:, 0, :]
                    dstt = v3t[:, 1, :]
                    nc.vector.tensor_tensor(out=srct, in0=srct, in1=dstt, op=mult)
                    nc.vector.tensor_tensor(
                        out=srct, in0=srct, in1=wb_tile[:tail, :], op=mult
                    )
                    redt = psmall.tile([tail, 1], F32, tag="redt")
                    nc.vector.tensor_reduce(
                        out=redt[:], in_=srct, axis=mybir.AxisListType.X, op=add
                    )
                    sigt = psmall.tile([tail, 1], F32, tag="sigt")
                    nc.scalar.activation(out=sigt[:], in_=redt[:], func=sig)
                    nc.sync.dma_start(
                        out=out_t[s, e_main:e].rearrange("(p w) -> p w", w=1),
                        in_=sigt[:],
                    )
    return nc


_PROGRAM = None


def _get_program():
    global _PROGRAM
    if _PROGRAM is None:
        _PROGRAM = build_program()
    return _PROGRAM


def make_in_maps(z, weight, edge_index):
    z = np.ascontiguousarray(np.asarray(z, dtype=np.float32))
    weight = np.asarray(weight, dtype=np.float32)
    ei = np.asarray(edge_index)
    t, _, e = ei.shape
    idx32 = ei.astype(np.int32).transpose(0, 2, 1)  # [T, E, 2] (edge-interleaved)
    wb = np.broadcast_to(weight[:, None, :], (t, 128, weight.shape[1]))
    in_maps = []
    for c in range(N_CORES):
        sl = slice(c * TPC, (c + 1) * TPC)
        in_maps.append(
            {
                "z": z,
                "wb": np.ascontiguousarray(wb[sl]),
                "idx": np.ascontiguousarray(idx32[sl]).reshape(TPC, 2 * e),
            }
        )
    return in_maps


def kernel(z, weight, edge_index, _trace=False):
    nc = _get_program()
    in_maps = make_in_maps(z, weight, edge_index)
    res = run_bass_kernel_spmd(
        nc, in_maps, core_ids=list(range(N_CORES)), trace=_trace
    )
    per_type = np.concatenate([res.results[c]["out"] for c in range(N_CORES)], axis=0)
    score = np.ascontiguousarray(per_type.reshape(-1))
    if _trace:
        kernel._last_results = res
    return per_type, score
